# revision 55
# baseline (speedup 1.0000x reference)
"""Trainium2 Bass kernel for e3nn-style BatchNorm (instance norm over graphs).

Problem: x [200000, 480] f32, irreps 128x0e + 64x1o + 32x2e, batch_id sorted
into 64 graphs, weight [224], bias [128].

Math (per graph g, derived from the reference):
  scalar block (cols 0:128, one col per channel c):
    m[g,c]   = mean_g(x_c)
    var[g,c] = mean_g(x_c^2) - m^2
    A[g,c]   = w_c / sqrt(var + eps);  B[g,c] = bias_c - m*A
    out      = x*A + B
  vector blocks (64 chans x dim 3, 32 chans x dim 5):
    fn[g,j]  = mean_g(mean_d(x^2))
    A[g,j]   = w_j / sqrt(fn + eps);  out = x*A

Sharding: 8 graphs per core (graph-aligned boundaries), rows padded so every
graph's row count is a multiple of RPP=8.  With the row mapping
row = g*1024 + p*8 + s, each SBUF partition p then holds rows of exactly ONE
graph -> the per-graph affine params become per-PARTITION params: the apply
gathers them with a single pair of hi/lo matmuls per 1024-row group (instead
of per-128-row subtile) and broadcasts along the slot dim with stride-0 APs.

Mixed-precision I/O (the kernel is memory-bound):
  - scalar cols 0:128 stay f32 (the (x-m)*A cancellation needs x to ~1e-5
    absolute near x~m; fp16's 2^-11 on |x|~0.07 would breach the 1e-3-floored
    rel-err gate),
  - vector cols ship as fp16 (pure scaling -> rel err 2^-11),
  - output ships as fp16 and is upcast on the host (rel err 2^-11 uniformly).
The scalar apply is sub-first: t = fp16(x - m_hat) then out = t*A, so every
fp16 rounding is relative to the output and only the gathered per-partition
mean m_hat needs the hi/lo fp16 gather split; A gathers as plain fp16 with
the A3/A5 sections pre-replicated along the irrep dim at params time, making
the whole vector-block apply one 2x-rate DVE multiply per group.

Stats: squares in fp16 (ACT), summed per-graph by one-hot matmuls (8 slots x
2 PSUM regions per group, shared lhsT slice of a precomputed one-hot tile)
accumulating IN PSUM across groups between ready points; sums spill to SBUF
only at segment ends.  1/count ships from the host (counts are final; params
of incomplete graphs are garbage-but-finite and never consumed).

Single-pass sliding window as before: batch_id is sorted, so group t can be
normalized and stored as soon as the stats pass has consumed the last row of
every graph touching it (host computes the f*(t) map, maxed across cores, and
the program is specialized to it).  Falls back to a two-pass program when the
needed window exceeds SBUF.
"""

import sys

if "/opt/trn_rl_repo" not in sys.path:
    sys.path.insert(0, "/opt/trn_rl_repo")

import numpy as np

P = 128          # partitions
RPP = 8          # consecutive rows per partition ("slots")
GROUP = P * RPP  # rows per group (1024)
CS = 128         # scalar cols (f32)
CV = 352         # vector cols (fp16)
CVW = CV         # no extra cols: counts ship from the host
C = 480          # data columns
STW = 610        # st tile: sq_s 0:128 | sq_v 128:480 | pad | x_s 482:610
NCORES = 8
G = 64           # total graphs
GPC = G // NCORES  # graphs per core
EPS = 1e-5
W_MAX = 12       # sliding-window tiles (stream path)
R_CACHE = 4      # two-pass fallback: trailing groups kept in SBUF
XT_BUFS = 4

_prog_cache = {}


def _setup(nc, bass, mybir, cp, ng):
    """Constant tiles shared by both builders."""
    f32 = mybir.dt.float32
    consts = {}
    # iota along free dim, bcast to all partitions: oh[p, g] = (bid == g)
    iota_t = cp.tile([P, GPC], f32, tag="iota_t")
    nc.gpsimd.dma_start(out=iota_t[:], in_=bass.AP(
        tensor=nc.t_iota, offset=0, ap=[[0, P], [1, GPC]]))
    iota_c = cp.tile([GPC, 1], f32, tag="iota_c")
    nc.gpsimd.dma_start(out=iota_c[:], in_=bass.AP(
        tensor=nc.t_iota, offset=0, ap=[[1, GPC], [1, 1]]))
    w_b = cp.tile([GPC, 224], f32, tag="w_b")
    nc.gpsimd.dma_start(out=w_b[:], in_=bass.AP(
        tensor=nc.t_w, offset=0, ap=[[0, GPC], [1, 224]]))
    bias_bP = cp.tile([P, 128], f32, tag="bias_bP")
    nc.gpsimd.dma_start(out=bias_bP[:], in_=bass.AP(
        tensor=nc.t_b, offset=0, ap=[[0, P], [1, 128]]))
    eps224 = cp.tile([GPC, 224], f32, tag="eps224")
    nc.vector.memset(eps224[:, 0:128], EPS)
    nc.vector.memset(eps224[:, 128:192], 3.0 * EPS)
    nc.vector.memset(eps224[:, 192:224], 5.0 * EPS)
    invc = cp.tile([GPC, 1], f32, tag="invc")
    nc.gpsimd.dma_start(out=invc[:], in_=bass.AP(
        tensor=nc.t_invc, offset=0, ap=[[1, GPC], [1, 1]]))
    # bid of row 8p of group t, in both layouts (values exact in f32),
    # expanded once into the one-hot tiles every group slices from
    f16 = mybir.dt.float16
    Alu = mybir.AluOpType
    bt_all = cp.tile([GPC, ng * P], f32, tag="bt_all")
    nc.gpsimd.dma_start(out=bt_all[:], in_=bass.AP(
        tensor=nc.t_bts, offset=0, ap=[[0, GPC], [1, ng * P]]))
    bt2T = cp.tile([P, ng], f32, tag="bt2T")
    nc.gpsimd.dma_start(out=bt2T[:], in_=bass.AP(
        tensor=nc.t_btsT, offset=0, ap=[[ng, P], [1, ng]]))
    oh2_all = cp.tile([GPC, ng * P], f16, tag="oh2_all")
    nc.vector.tensor_scalar(out=oh2_all[:], in0=bt_all[:],
                            scalar1=iota_c[:], scalar2=None,
                            op0=Alu.is_equal)
    oh_all = cp.tile([P, ng, GPC], f16, tag="oh_all")
    b2 = bt2T[:]
    it = iota_t[:]
    nc.vector.tensor_tensor(
        out=oh_all[:],
        in0=bass.AP(tensor=b2.tensor, offset=b2.offset,
                    ap=[b2.ap[0], [1, ng], [0, GPC]]),
        in1=bass.AP(tensor=it.tensor, offset=it.offset,
                    ap=[it.ap[0], [0, ng], [1, GPC]]),
        op=Alu.is_equal)
    consts.update(iota_t=iota_t, iota_c=iota_c, w_b=w_b, bias_bP=bias_bP,
                  eps224=eps224, invc=invc, oh2_all=oh2_all, oh_all=oh_all)
    return consts


def _declare_io(nc, mybir, n_pad):
    f32 = mybir.dt.float32
    f16 = mybir.dt.float16
    nc.t_xs = nc.dram_tensor("xs", [n_pad, CS], f32, kind="ExternalInput")
    nc.t_xv = nc.dram_tensor("xv", [n_pad, CVW], f16, kind="ExternalInput")
    ng = n_pad // GROUP
    nc.t_bts = nc.dram_tensor("bts", [ng * P], f32, kind="ExternalInput")
    nc.t_btsT = nc.dram_tensor("btsT", [P * ng], f32, kind="ExternalInput")
    nc.t_invc = nc.dram_tensor("invc", [GPC], f32, kind="ExternalInput")
    nc.t_iota = nc.dram_tensor("iota8", [GPC], f32, kind="ExternalInput")
    nc.t_w = nc.dram_tensor("w", [224], f32, kind="ExternalInput")
    nc.t_b = nc.dram_tensor("b", [128], f32, kind="ExternalInput")
    nc.t_out = nc.dram_tensor("out", [n_pad, C], f16, kind="ExternalOutput")
    # row (g*GROUP + p*RPP + s) -> [g][p][s]: each partition holds RPP
    # consecutive rows of one graph (host pads graphs to RPP multiples)
    xs_g = nc.t_xs.ap().rearrange("(g p r) c -> g p r c", p=P, r=RPP)
    xv_g = nc.t_xv.ap().rearrange("(g p r) c -> g p r c", p=P, r=RPP)
    out_g = nc.t_out.ap().rearrange("(g p r) c -> g p r c", p=P, r=RPP)
    return xs_g, xv_g, out_g


def _phase1_ops(nc, bass, mybir, sqp, t, xst, xvt, cs):
    """Squares/copy for one group; returns (st, oh lhsT slice)."""
    f16 = mybir.dt.float16
    Act = mybir.ActivationFunctionType
    st = sqp.tile([P, RPP, STW], f16, tag="st")
    nc.scalar.activation(out=st[:, :, 0:CS], in_=xst[:], func=Act.Square)
    nc.scalar.activation(out=st[:, :, CS:C], in_=xvt[:], func=Act.Square)
    nc.vector.tensor_copy(out=st[:, :, 482:STW], in_=xst[:])
    return st, cs["oh_all"][:, t, :]


def _stats_matmuls(nc, p_all, oh, st, first, last):
    """8 slot matmuls x 2 PSUM regions, accumulating into p_all."""
    # one matmul per slot covering sq | junk(480:482) | x-copy; the out
    # region spans two PSUM banks (2440B)
    for s in range(RPP):
        stf = first and s == 0
        spf = last and s == RPP - 1
        nc.tensor.matmul(out=p_all[:, 0:STW], lhsT=oh,
                         rhs=st[:, s, 0:STW], start=stf, stop=spf)


def _params_ops(nc, bass, mybir, cp, acc, cs):
    """Affine params from accumulated sums; returns fp16 (par_h, par_l).

    acc layout [8, 610] f32: 0:128 sum xs^2 | 128:480 sum xv^2 |
    480:482 junk | 482:610 sum xs.  1/count ships from the host (final
    counts; incomplete graphs' params are garbage-but-finite either way).
    params layout [8, 608]: m 0:128 | A_s 128:256 | A3x3 256:448 | A5x5
    448:608 (A3/A5 pre-replicated along the irrep dim so the apply's
    per-partition scale is contiguous).  Only the m section needs the fp16
    hi/lo split (par_l covers cols 0:128); the A sections ride par_h alone.
    Entries for incomplete graphs are garbage but kept finite (counts
    clamped >= 1, fn clamped >= 0); ready rows never reference them.
    """
    f32 = mybir.dt.float32
    f16 = mybir.dt.float16
    Alu = mybir.AluOpType
    Act = mybir.ActivationFunctionType
    t = lambda shape, dt, name: cp.tile(shape, dt, tag=name, name=name)

    em = t([GPC, 610], f32, "em")
    nc.vector.tensor_scalar_mul(out=em[:], in0=acc[:], scalar1=cs["invc"][:])

    fn = t([GPC, 224], f32, "fn")
    m2 = t([GPC, 128], f32, "m2")
    nc.vector.tensor_tensor(out=m2[:], in0=em[:, 482:610], in1=em[:, 482:610],
                            op=Alu.mult)
    nc.vector.tensor_tensor(out=fn[:, 0:128], in0=em[:, 0:128], in1=m2[:],
                            op=Alu.subtract)
    nc.vector.tensor_reduce(out=fn[:, 128:192],
                            in_=em[:, 128:320].rearrange(
                                "p (j d) -> p j d", d=3),
                            axis=mybir.AxisListType.X, op=Alu.add)
    nc.vector.tensor_reduce(out=fn[:, 192:224],
                            in_=em[:, 320:480].rearrange(
                                "p (j d) -> p j d", d=5),
                            axis=mybir.AxisListType.X, op=Alu.add)
    nc.vector.tensor_scalar_max(out=fn[:], in0=fn[:], scalar1=0.0)
    nc.vector.tensor_tensor(out=fn[:], in0=fn[:], in1=cs["eps224"][:],
                            op=Alu.add)
    # rstd = 1/sqrt(fn); w for the 3/5 sections is pre-scaled by sqrt(d) and
    # eps by d on the host side of the fold (see eps224 memsets)
    sqv = t([GPC, 224], f32, "sqv")
    nc.scalar.activation(out=sqv[:], in_=fn[:], func=Act.Sqrt)
    nc.vector.reciprocal_approx_fast(out=fn[:], in_=sqv[:])

    def _exp(ap_in, j, d):
        return bass.AP(tensor=ap_in.tensor, offset=ap_in.offset,
                       ap=[ap_in.ap[0], [1, j], [0, d]])

    params = t([GPC, 608], f32, "params")
    nc.vector.tensor_copy(out=params[:, 0:128], in_=em[:, 482:610])
    nc.vector.tensor_tensor(out=params[:, 128:256], in0=fn[:, 0:128],
                            in1=cs["w_b"][:, 0:128], op=Alu.mult)
    nc.vector.tensor_tensor(
        out=params[:, 256:448].rearrange("p (j d) -> p j d", d=3),
        in0=_exp(fn[:, 128:192], 64, 3), in1=_exp(cs["w_b"][:, 128:192],
                                                  64, 3), op=Alu.mult)
    nc.vector.tensor_tensor(
        out=params[:, 448:608].rearrange("p (j d) -> p j d", d=5),
        in0=_exp(fn[:, 192:224], 32, 5), in1=_exp(cs["w_b"][:, 192:224],
                                                  32, 5), op=Alu.mult)

    par_h = cp.tile([GPC, 608], f16, tag="par_h")
    nc.vector.tensor_copy(out=par_h[:], in_=params[:])
    ph32 = t([GPC, 128], f32, "ph32")
    nc.vector.tensor_copy(out=ph32[:], in_=par_h[:, 0:128])
    par_l = cp.tile([GPC, 128], f16, tag="par_l")
    nc.vector.tensor_tensor(out=par_l[:], in0=params[:, 0:128], in1=ph32[:],
                            op=Alu.subtract)
    return par_h, par_l


def _bc(bass, a):
    """Broadcast a [P, n] AP along a middle slot dim of size RPP."""
    return bass.AP(tensor=a.tensor, offset=a.offset,
                   ap=[a.ap[0], [0, RPP], a.ap[1]])


def _apply_ops(nc, bass, mybir, ohp, otp, tsp, ps2, out_g, g, xst, xvt,
               par_h, par_l, cs, ring, bias_zero, sub_eng, drain=False):
    """Gather params for group g (per-partition) and write normalized out.

    Scalar block: t = fp16(x - m_hat) then out = t * A (all rounding is
    relative to the output, so fp16 is safe even at the 1e-3 denom floor);
    m_hat comes through the hi/lo gather at ~1e-5 absolute.
    """
    f32 = mybir.dt.float32
    f16 = mybir.dt.float16
    Alu = mybir.AluOpType
    Act = mybir.ActivationFunctionType
    oh2 = cs["oh2_all"][:, g * P:(g + 1) * P]
    gp = ps2.tile([P, 640], f32, tag="gp")
    nc.tensor.matmul(out=gp[:, 0:128], lhsT=oh2, rhs=par_h[:, 0:128],
                     start=True, stop=False)
    nc.tensor.matmul(out=gp[:, 0:128], lhsT=oh2, rhs=par_l[:],
                     start=False, stop=True)
    nc.tensor.matmul(out=gp[:, 128:512], lhsT=oh2, rhs=par_h[:, 128:512],
                     start=True, stop=True)
    nc.tensor.matmul(out=gp[:, 512:608], lhsT=oh2, rhs=par_h[:, 512:608],
                     start=True, stop=True)
    gm = ohp.tile([P, 128], f32, tag="gm")
    nc.scalar.activation(out=gm[:], in_=gp[:, 0:128], func=Act.Copy)
    ga = ohp.tile([P, 480], f16, tag="ga")
    nc.scalar.activation(out=ga[:], in_=gp[:, 128:608], func=Act.Copy)

    ot = otp.tile([P, RPP, C], f16, tag="ot")
    if bias_zero:
        ts = tsp.tile([P, RPP, 128], f16, tag="ts")
        sub_eng.tensor_tensor(out=ts[:], in0=xst[:],
                              in1=_bc(bass, gm[:]), op=Alu.subtract)
        # drain: keep the mult on the sub's engine (no cross-engine hop;
        # sheds drain-DVE work onto the 28%-busy gpsimd)
        mul_eng = sub_eng if drain else nc.vector
        mul_eng.tensor_tensor(out=ot[:, :, 0:128], in0=ts[:],
                              in1=_bc(bass, ga[:, 0:128]), op=Alu.mult)
    else:
        # (x - m)*A + b needs the intermediates in f32: near out ~ 0 the
        # cancellation against b would amplify any fp16 rounding
        ts = tsp.tile([P, RPP, 128], f32, tag="ts32")
        sub_eng.tensor_tensor(out=ts[:], in0=xst[:],
                              in1=_bc(bass, gm[:]), op=Alu.subtract)
        nc.vector.tensor_tensor(out=ts[:], in0=ts[:],
                                in1=_bc(bass, ga[:, 0:128]), op=Alu.mult)
        nc.vector.tensor_tensor(out=ot[:, :, 0:128], in0=ts[:],
                                in1=_bc(bass, cs["bias_bP"][:]), op=Alu.add)
    nc.vector.tensor_tensor(out=ot[:, :, 128:480], in0=xvt[:, :, 0:CV],
                            in1=_bc(bass, ga[:, 128:480]), op=Alu.mult)
    ring.dma_start(out=out_g[g], in_=ot[:])


def _build_stream(n_pad, fstar, bias_zero):
    """Single-pass sliding-window program."""
    import concourse.bacc as bacc
    import concourse.bass as bass
    import concourse.tile as tile
    from concourse import mybir

    f32 = mybir.dt.float32
    Alu = mybir.AluOpType
    ng = n_pad // GROUP
    applies_at = {}
    for t, f in enumerate(fstar):
        applies_at.setdefault(f, []).append(t)
    seg_ends = sorted(set(applies_at) | {ng - 1})

    nc = bacc.Bacc("TRN2", target_bir_lowering=False, debug=False,
                   num_devices=NCORES)
    xs_g, xv_g, out_g = _declare_io(nc, mybir, n_pad)

    with tile.TileContext(nc) as tc:
        with (
            tc.tile_pool(name="const", bufs=1) as cp,
            tc.tile_pool(name="par", bufs=2) as pp,
            tc.tile_pool(name="xs", bufs=W_MAX) as xsp,
            tc.tile_pool(name="xv", bufs=W_MAX) as xvp,
            tc.tile_pool(name="sq", bufs=2) as sqp,
            tc.tile_pool(name="oh", bufs=2) as ohp,
            tc.tile_pool(name="ot", bufs=3) as otp,
            tc.tile_pool(name="tsb", bufs=2) as tsp,
            tc.tile_pool(name="ps1", bufs=2, space="PSUM") as ps1,
            tc.tile_pool(name="ps2", bufs=2, space="PSUM") as ps2,
        ):
            cs = _setup(nc, bass, mybir, cp, ng)
            acc = cp.tile([GPC, 610], f32, tag="acc")
            nc.vector.memset(acc[:], 0.0)

            xts = {}
            p_all = None
            seg_start = True
            for t in range(ng):
                xst = xsp.tile([P, RPP, CS], f32, tag="xs")
                xvt = xvp.tile([P, RPP, CVW], mybir.dt.float16, tag="xv")
                xts[t] = (xst, xvt)
                nc.sync.dma_start(out=xst[:], in_=xs_g[t])
                nc.sync.dma_start(out=xvt[:], in_=xv_g[t])
                st, oh = _phase1_ops(nc, bass, mybir, sqp, t, xst, xvt, cs)
                if seg_start:
                    p_all = ps1.tile([GPC, 610], f32, tag="p_all")
                seg_end = t in seg_ends
                _stats_matmuls(nc, p_all, oh, st, seg_start, seg_end)
                seg_start = seg_end
                if seg_end:
                    nc.vector.tensor_tensor(out=acc[:], in0=acc[:],
                                            in1=p_all[:], op=Alu.add)
                if t in applies_at:
                    par_h, par_l = _params_ops(nc, bass, mybir, pp, acc, cs)
                    drain = len(seg_ends) > 1 and t >= seg_ends[-2]
                    for i, tp in enumerate(applies_at[t]):
                        ring = nc.scalar if i % 2 == 0 else nc.sync
                        # post-last-load drain: split sub+mult chains whole
                        # across both engines (no cross-engine hop inside
                        # a chain)
                        sub_eng = (nc.vector if drain and i % 2 == 1
                                   else nc.gpsimd)
                        xst_a, xvt_a = xts.pop(tp)
                        _apply_ops(nc, bass, mybir, ohp, otp, tsp, ps2,
                                   out_g, tp, xst_a, xvt_a, par_h, par_l,
                                   cs, ring, bias_zero, sub_eng, drain)

    nc.compile()
    return nc


def _build_twopass(n_pad, bias_zero):
    """Fallback: stats pass + re-read apply pass (bounded SBUF window)."""
    import concourse.bacc as bacc
    import concourse.bass as bass
    import concourse.tile as tile
    from concourse import mybir

    f32 = mybir.dt.float32
    Alu = mybir.AluOpType
    ng = n_pad // GROUP
    r_cache = min(R_CACHE, ng)

    nc = bacc.Bacc("TRN2", target_bir_lowering=False, debug=False,
                   num_devices=NCORES)
    xs_g, xv_g, out_g = _declare_io(nc, mybir, n_pad)

    with tile.TileContext(nc) as tc:
        with (
            tc.tile_pool(name="const", bufs=1) as cp,
            tc.tile_pool(name="par", bufs=1) as pp,
            tc.tile_pool(name="xs", bufs=XT_BUFS) as xsp,
            tc.tile_pool(name="xv", bufs=XT_BUFS) as xvp,
            tc.tile_pool(name="xsc", bufs=max(r_cache, 1)) as xscp,
            tc.tile_pool(name="xvc", bufs=max(r_cache, 1)) as xvcp,
            tc.tile_pool(name="sq", bufs=2) as sqp,
            tc.tile_pool(name="oh", bufs=2) as ohp,
            tc.tile_pool(name="ot", bufs=3) as otp,
            tc.tile_pool(name="tsb", bufs=2) as tsp,
            tc.tile_pool(name="ps1", bufs=1, space="PSUM") as ps1,
            tc.tile_pool(name="ps2", bufs=2, space="PSUM") as ps2,
        ):
            cs = _setup(nc, bass, mybir, cp, ng)
            acc = cp.tile([GPC, 610], f32, tag="acc")
            nc.vector.memset(acc[:], 0.0)

            cached = {}
            p_all = ps1.tile([GPC, 610], f32, tag="p_all")
            for g in range(ng):
                if g >= ng - r_cache:
                    xst = xscp.tile([P, RPP, CS], f32, tag="xsc")
                    xvt = xvcp.tile([P, RPP, CVW], mybir.dt.float16,
                                    tag="xvc")
                    cached[g] = (xst, xvt)
                else:
                    xst = xsp.tile([P, RPP, CS], f32, tag="xs")
                    xvt = xvp.tile([P, RPP, CVW], mybir.dt.float16, tag="xv")
                nc.sync.dma_start(out=xst[:], in_=xs_g[g])
                nc.sync.dma_start(out=xvt[:], in_=xv_g[g])
                st, oh = _phase1_ops(nc, bass, mybir, sqp, g, xst, xvt, cs)
                _stats_matmuls(nc, p_all, oh, st, g == 0, g == ng - 1)
            nc.vector.tensor_tensor(out=acc[:], in0=acc[:],
                                    in1=p_all[:], op=Alu.add)

            par_h, par_l = _params_ops(nc, bass, mybir, pp, acc, cs)
            for g in range(ng):
                if g in cached:
                    xst, xvt = cached[g]
                else:
                    xst = xsp.tile([P, RPP, CS], f32, tag="xs")
                    xvt = xvp.tile([P, RPP, CVW], mybir.dt.float16, tag="xv")
                    nc.sync.dma_start(out=xst[:], in_=xs_g[g])
                    nc.sync.dma_start(out=xvt[:], in_=xv_g[g])
                ring = nc.scalar if g % 2 == 0 else nc.sync
                _apply_ops(nc, bass, mybir, ohp, otp, tsp, ps2, out_g, g,
                           xst, xvt, par_h, par_l, cs, ring, bias_zero,
                           nc.gpsimd)

    nc.compile()
    return nc


def kernel(input, batch_id_tensor, weight, bias, _trace=False):
    from concourse import bass_utils

    x = np.ascontiguousarray(np.asarray(input, dtype=np.float32))
    bid = np.asarray(batch_id_tensor).astype(np.int64)
    w = np.asarray(weight, dtype=np.float32).copy()
    b = np.asarray(bias, dtype=np.float32)
    n = x.shape[0]
    # fold the 1/d of the component mean into w (and eps, on-device)
    w[128:192] *= np.sqrt(3.0)
    w[192:224] *= np.sqrt(5.0)

    # graph-aligned core boundaries; each graph padded to a multiple of RPP
    glo = np.searchsorted(bid, np.arange(G + 1), side="left")
    cnt = np.diff(glo)
    padded = -(-cnt // RPP) * RPP
    core_rows = padded.reshape(NCORES, GPC).sum(axis=1)
    n_pad = max(GROUP, int(-(-core_rows.max() // GROUP)) * GROUP)
    ng = n_pad // GROUP

    # per-core padded bid arrays (pad rows carry their graph id; tail = GPC)
    bid_pad = np.full((NCORES, n_pad), float(GPC), dtype=np.float32)
    starts = np.zeros((NCORES, GPC), dtype=np.int64)
    for c in range(NCORES):
        pos = 0
        for j in range(GPC):
            g = c * GPC + j
            starts[c, j] = pos
            bid_pad[c, pos:pos + padded[g]] = float(j)
            pos += padded[g]

    # f*(t): first group index by which every graph touching group t is
    # fully consumed, maxed across cores so one SPMD program serves all
    fstar = np.arange(ng)
    for c in range(NCORES):
        ends = starts[c] + padded[c * GPC:(c + 1) * GPC]  # padded graph ends
        nrows = int(ends[-1])
        for t in range(ng):
            last = min((t + 1) * GROUP, nrows) - 1
            if last < t * GROUP:
                continue
            gl = int(np.searchsorted(ends, last, side="right"))
            end_row = int(ends[gl]) - 1 if gl < GPC else nrows - 1
            fstar[t] = max(fstar[t], end_row // GROUP)
    w_need = int((fstar - np.arange(ng)).max()) + 1

    bias_zero = bool(np.all(b == 0.0))
    if w_need + 1 <= W_MAX:
        key = (n_pad, tuple(int(f) for f in fstar), bias_zero)
        if key not in _prog_cache:
            _prog_cache[key] = _build_stream(n_pad, tuple(fstar), bias_zero)
    else:
        key = (n_pad, None, bias_zero)
        if key not in _prog_cache:
            _prog_cache[key] = _build_twopass(n_pad, bias_zero)
    nc = _prog_cache[key]

    iota = np.arange(GPC, dtype=np.float32)
    xv_full = x[:, CS:C].astype(np.float16)
    in_maps = []
    for c in range(NCORES):
        xs = np.zeros((n_pad, CS), dtype=np.float32)
        xv = np.zeros((n_pad, CVW), dtype=np.float16)
        for j in range(GPC):
            g = c * GPC + j
            lo, hi = int(glo[g]), int(glo[g + 1])
            p0 = int(starts[c, j])
            xs[p0:p0 + cnt[g]] = x[lo:hi, 0:CS]
            xv[p0:p0 + cnt[g]] = xv_full[lo:hi]
        bts = np.ascontiguousarray(bid_pad[c][::RPP])
        invc = 1.0 / np.maximum(
            cnt[c * GPC:(c + 1) * GPC].astype(np.float32), 1.0)
        in_maps.append({
            "xs": xs,
            "xv": xv,
            "bts": bts,
            "btsT": np.ascontiguousarray(bts.reshape(ng, P).T).ravel(),
            "invc": invc,
            "iota8": iota,
            "w": w,
            "b": b,
        })

    res = bass_utils.run_bass_kernel_spmd(
        nc, in_maps, core_ids=list(range(NCORES)), trace=_trace)

    out = np.empty((n, C), dtype=np.float32)
    for c in range(NCORES):
        oc = res.results[c]["out"]
        for j in range(GPC):
            g = c * GPC + j
            lo, hi = int(glo[g]), int(glo[g + 1])
            p0 = int(starts[c, j])
            out[lo:hi] = oc[p0:p0 + cnt[g]]
    if _trace:
        return out, res
    return out
